# revision 11
# baseline (speedup 1.0000x reference)
"""Trainium2 Bass kernel for the hard-negative-mining set loss (v7).

Key structural changes vs v6:
  * mining rows == member rows: core k mines the M=8 member rows of its
    128 owned classes (any row->core partition works since keys carry
    the true global row id and combine by max). xloc/xmem unify -> one
    2MB bf16 load instead of 12MB f32.
  * no combo tensor: under this sharding the same-class mask column is
    the partition's own class for every tile, so a single eqmcs
    (-2^24 at col g_p) is added ONCE to the maxed key accumulator; the
    enc tiebreak rides the per-partition tensor_scalar scalars.
  * tail: rsum_m = sum exp(mem_m + mem_0 + neg - 14) computed by a DVE
    add + fused ACT exp-accumulate (no Pool multiplies / reduces).
  * negrow gathered from a bf16 copy of x in one indirect DMA.
"""

import ml_dtypes
import numpy as np

import concourse.bass as bass
import concourse.bacc as bacc
import concourse.tile as tile
from concourse import mybir
from concourse import bass_isa
from concourse.bass_utils import run_bass_kernel_spmd
from concourse.tile import add_dep_helper

B, C = 8192, 1024
NCORES = 8
CCL = C // NCORES     # 128 classes owned per core
M = B // C            # 8 members per class

SHIFT_A = 10.0        # mining softmax shift
SHIFT_C = 14.0        # summed-logits softmax shift
QSCALE = 140.0        # log-prob quantization: 1/140 nat resolution
SCALE = QSCALE * 8192.0            # 1146880.0
M2 = 1.5 * (2.0 ** 36)             # magic: ulp(M2) = 8192
M2C = M2 + SCALE * SHIFT_A         # exact multiple of 8192
MASKC = -16777216.0                # -2^24 same-class exclusion
F32 = mybir.dt.float32
BF16 = mybir.dt.bfloat16
I32 = mybir.dt.int32
OP = mybir.AluOpType
AF = mybir.ActivationFunctionType
AX = mybir.AxisListType


def build_nc():
    nc = bacc.Bacc("TRN2", target_bir_lowering=False, debug=False,
                   num_devices=NCORES)

    xbf_d = nc.dram_tensor("xbf", [B, C], BF16, kind="ExternalInput")
    xmem_d = nc.dram_tensor("xmem", [M * 128, C], BF16, kind="ExternalInput")
    eqmcs_d = nc.dram_tensor("eqmcs", [128, C], F32, kind="ExternalInput")
    encm_d = nc.dram_tensor("encm", [128, M], F32, kind="ExternalInput")
    dxm_d = nc.dram_tensor("dxm", [128, M], F32, kind="ExternalInput")
    w8_d = nc.dram_tensor("w8", [128, M], F32, kind="ExternalInput")
    w7_d = nc.dram_tensor("w7", [128, M - 1], F32, kind="ExternalInput")
    out_d = nc.dram_tensor("partial", [1, 1], F32, kind="ExternalOutput")

    cc_in = nc.dram_tensor("cc_in", [1, C], F32)
    cc_out = nc.dram_tensor("cc_out", [1, C], F32)

    with tile.TileContext(nc) as tc:
        with (
            tc.tile_pool(name="persist", bufs=1) as pp,
            tc.tile_pool(name="rscr", bufs=3) as rp,
            tc.tile_pool(name="kscr", bufs=3) as kp,
            tc.tile_pool(name="tscr", bufs=2) as tp,
            tc.tile_pool(name="small", bufs=6) as smp,
            tc.tile_pool(name="psB", bufs=1, space="PSUM") as psb,
        ):
            # ---------- input DMAs ----------
            xm = []
            for m in range(M):
                xt = pp.tile([128, C], BF16, tag=f"xm{m}")
                nc.gpsimd.dma_start(out=xt, in_=xmem_d.ap()[m * 128:(m + 1) * 128, :])
                xm.append(xt)
            # aux DMAs on the scalar HWDGE queue: keeps the sync queue free
            # for the runtime's ACT-table staging (gates the first exp)
            eqmcs = pp.tile([128, C], F32, tag="eqmcs")
            nc.scalar.dma_start(out=eqmcs, in_=eqmcs_d.ap())
            encm = pp.tile([128, M], F32, tag="encm")
            nc.scalar.dma_start(out=encm, in_=encm_d.ap())
            dxm = pp.tile([128, M], F32, tag="dxm")
            nc.scalar.dma_start(out=dxm, in_=dxm_d.ap())
            w8 = pp.tile([128, M], F32, tag="w8")
            nc.scalar.dma_start(out=w8, in_=w8_d.ap())
            w7 = pp.tile([128, M - 1], F32, tag="w7")
            nc.scalar.dma_start(out=w7, in_=w7_d.ap())

            ones = pp.tile([128, 1], F32, tag="ones")
            nc.vector.memset(ones, 1.0)
            shA = pp.tile([128, 1], F32, tag="shA")
            nc.vector.memset(shA, -SHIFT_A)
            shC = pp.tile([128, 1], F32, tag="shC")
            nc.vector.memset(shC, -SHIFT_C)

            # ---------- mining: packed-key build ----------
            # rs[:, m] = sum_c exp(xm_m - SHIFT_A)
            dumpA = pp.tile([128, C], BF16, tag="dumpA")
            rscat = smp.tile([128, M], F32, tag="rscat")
            for m in range(M):
                nc.scalar.activation(out=dumpA, in_=xm[m], func=AF.Exp,
                                     bias=shA, scale=1.0,
                                     accum_out=rscat[:, m:m + 1])
            lrcat = smp.tile([128, M], F32, tag="lrcat")
            ln_ins = nc.scalar.activation(out=lrcat, in_=rscat, func=AF.Ln)
            # b_m = f32(SCALE*lr + M2C): multiple of 8192 (carries lnrsum)
            btcat = smp.tile([128, M], F32, tag="btcat")
            nc.vector.tensor_scalar(out=btcat, in0=lrcat, scalar1=SCALE,
                                    scalar2=M2C, op0=OP.mult, op1=OP.add)
            bt = [btcat[:, m:m + 1] for m in range(M)]
            en = [encm[:, m:m + 1] for m in range(M)]
            # r_m = Relu(-SCALE*x + b_m) = b_m + 8192*q (fp32 rounds @8192)
            # K_m = (r_m - b_m) + enc_m ; Kacc = max_m K_m ; then += eqmcs
            # two max-accumulator chains (m 0-3 / 4-7) to shorten the
            # serial dependency behind the relu stream
            KaccA = pp.tile([128, C], F32, tag="KaccA")
            KaccB = pp.tile([128, C], F32, tag="KaccB")
            acc = {0: KaccA, 4: KaccB}
            relu_ins = None
            HC = C // 2
            for m in range(M):
                rt = rp.tile([128, C], F32, tag="relu")
                relu_ins = nc.scalar.activation(out=rt, in_=xm[m],
                                                func=AF.Relu,
                                                bias=bt[m], scale=-SCALE)
                if m == 0:
                    # pin ACT queue order: the Ln must precede the relus
                    add_dep_helper(relu_ins.ins, ln_ins.ins, sync=False)
                dst = KaccA if m < 4 else KaccB
                if m in acc:
                    nc.vector.tensor_scalar(
                        out=dst, in0=rt, scalar1=bt[m], op0=OP.subtract,
                        scalar2=en[m], op1=OP.add)
                elif m < M - 1:
                    kt = kp.tile([128, C], F32, tag="kt")
                    nc.vector.tensor_scalar(
                        out=kt, in0=rt, scalar1=bt[m], op0=OP.subtract,
                        scalar2=en[m], op1=OP.add)
                    nc.vector.scalar_tensor_tensor(
                        out=dst, in0=dst, scalar=0.0, op0=OP.add,
                        in1=kt, op1=OP.max)
                else:
                    # last tile: column-halved so the first merge/mask/
                    # partition-reduce half starts ~2.5us earlier and
                    # pipelines against the second half's DVE work
                    kt = kp.tile([128, C], F32, tag="kt")
                    for h in (0, 1):
                        lo, hi = h * HC, (h + 1) * HC
                        nc.vector.tensor_scalar(
                            out=kt[:, lo:hi], in0=rt[:, lo:hi],
                            scalar1=bt[m], op0=OP.subtract,
                            scalar2=en[m], op1=OP.add)
                        nc.vector.scalar_tensor_tensor(
                            out=dst[:, lo:hi], in0=dst[:, lo:hi], scalar=0.0,
                            op0=OP.add, in1=kt[:, lo:hi], op1=OP.max)

            # merge chains + same-class exclusion + per-partition reduce +
            # cc_in store, all column-halved and interleaved: DVE half 1
            # overlaps Pool's partition-reduce of half 0
            Kacc = pp.tile([128, C], F32, tag="Kacc")
            Kpar = pp.tile([128, C], F32, tag="Kpar")
            last_k = None
            for h in (0, 1):
                lo, hi = h * HC, (h + 1) * HC
                nc.vector.scalar_tensor_tensor(
                    out=Kacc[:, lo:hi], in0=KaccA[:, lo:hi], scalar=0.0,
                    op0=OP.add, in1=KaccB[:, lo:hi], op1=OP.max)
                last_k = nc.vector.tensor_tensor(
                    out=Kacc[:, lo:hi], in0=Kacc[:, lo:hi],
                    in1=eqmcs[:, lo:hi], op=OP.add)
                nc.gpsimd.partition_all_reduce(
                    out_ap=Kpar[:, lo:hi], in_ap=Kacc[:, lo:hi], channels=128,
                    reduce_op=bass_isa.ReduceOp.max)
                nc.scalar.dma_start(out=cc_in.ap()[0:1, lo:hi],
                                    in_=Kpar[0:1, lo:hi])
            # 4KB AllToAll: my out chunk n = core n's local max for MY
            # 128 classes
            nc.gpsimd.collective_compute(
                "AllToAll", OP.bypass,
                replica_groups=[list(range(NCORES))],
                ins=[cc_in.ap().opt()], outs=[cc_out.ap().opt()])

            # ---------- overlapped with barrier/exchange ----------
            # E_m = exp(mem_m + mem_0 - SHIFT_C), bf16, for the tail dots
            Em = []
            for m in range(1, M):
                st = rp.tile([128, C], F32, tag="Sm")
                s_ins = nc.vector.scalar_tensor_tensor(
                    out=st, in0=xm[m], scalar=0.0, op0=OP.add,
                    in1=xm[0], op1=OP.add)
                et = pp.tile([128, C], BF16, tag=f"Em{m}")
                e_ins = nc.scalar.activation(out=et, in_=st, func=AF.Exp,
                                             bias=shC, scale=1.0)
                Em.append(et)
                if m == 1:
                    add_dep_helper(s_ins.ins, last_k.ins, sync=False)
                    add_dep_helper(e_ins.ins, relu_ins.ins, sync=False)
            # tsh = sum_m w8_m * dxm_m  (host-gathered target logits)
            scr8 = smp.tile([128, M], F32, tag="scr8")
            tsh = smp.tile([128, 1], F32, tag="tsh")
            nc.vector.scalar_tensor_tensor(
                out=scr8, in0=dxm, scalar=1.0, in1=w8,
                op0=OP.mult, op1=OP.mult, accum_out=tsh)

            # ---------- combine + decode + neg gather ----------
            # cc_out chunk s = core s's local max for MY classes; load it
            # as [128, 8] (partition p = my class p, col s = sender) and
            # max-reduce over the free dim -> global key per owned class
            k8t = smp.tile([CCL, NCORES], F32, tag="k8t")
            k8src = bass.AP(tensor=cc_out.ap().tensor, offset=0,
                            ap=[[1, CCL], [CCL, NCORES]])
            nc.scalar.dma_start(out=k8t, in_=k8src)
            gk = smp.tile([128, 1], F32, tag="gk")
            nc.vector.tensor_reduce(out=gk, in_=k8t, axis=AX.X, op=OP.max)
            # decode: key is an exact integer in f32; row = 8191 - (key & 8191)
            gi = smp.tile([128, 1], I32, tag="gi")
            nc.vector.tensor_copy(out=gi, in_=gk)
            enc_i = smp.tile([128, 1], I32, tag="enc_i")
            nc.vector.tensor_scalar(out=enc_i, in0=gi, scalar1=8191,
                                    scalar2=None, op0=OP.bitwise_and)
            rowi = smp.tile([128, 1], I32, tag="rowi")
            nc.vector.tensor_scalar(out=rowi, in0=enc_i, scalar1=-1,
                                    scalar2=8191, op0=OP.mult, op1=OP.add)
            # ---------- factored CE tail ----------
            # N = exp(neg); rsum_m = sum_c E_m * N   (bf16 dots, f32 accum)
            # gather + N-exp in column halves: the exp of half 0 overlaps
            # the transfer of half 1
            negrow = pp.tile([128, C], BF16, tag="negrow")
            Nt = pp.tile([128, C], BF16, tag="Nt")
            n_ins = None
            for h in (0, 1):
                lo, hi = h * HC, (h + 1) * HC
                nc.gpsimd.indirect_dma_start(
                    out=negrow[:, lo:hi], out_offset=None,
                    in_=xbf_d.ap(), element_offset=lo,
                    in_offset=bass.IndirectOffsetOnAxis(ap=rowi[:, 0:1],
                                                        axis=0))
                n_ins = nc.scalar.activation(out=Nt[:, lo:hi],
                                             in_=negrow[:, lo:hi],
                                             func=AF.Exp, bias=0.0, scale=1.0)
                if h == 0:
                    add_dep_helper(n_ins.ins, e_ins.ins, sync=False)
            rscat3 = smp.tile([128, M - 1], F32, tag="rscat3")
            dumpC = tp.tile([128, C], BF16, tag="dumpC")
            for m in range(1, M):
                nc.vector.scalar_tensor_tensor(
                    out=dumpC, in0=Em[m - 1], scalar=1.0, op0=OP.mult,
                    in1=Nt, op1=OP.mult,
                    accum_out=rscat3[:, m - 1:m])
            # dneg_scaled = -2^24 * neg[p, class(p)] via eqmcs diag dot
            scrC = tp.tile([128, C], F32, tag="scrC")
            dnegs = smp.tile([128, 1], F32, tag="dnegs")
            nc.vector.scalar_tensor_tensor(out=scrC, in0=negrow, scalar=1.0,
                                           in1=eqmcs, op0=OP.mult, op1=OP.mult,
                                           accum_out=dnegs)
            lcat = smp.tile([128, M - 1], F32, tag="lcat")
            lcat_ins = nc.scalar.activation(out=lcat, in_=rscat3, func=AF.Ln)
            add_dep_helper(lcat_ins.ins, n_ins.ins, sync=False)
            scr7 = smp.tile([128, M - 1], F32, tag="scr7")
            wl = smp.tile([128, 1], F32, tag="wl")
            nc.vector.scalar_tensor_tensor(out=scr7, in0=lcat, scalar=1.0,
                                           in1=w7, op0=OP.mult, op1=OP.mult,
                                           accum_out=wl)
            # loss_p = wl + 8*SHIFT_C - tsh - 8*dneg
            #        = wl + 8*SHIFT_C - tsh + dnegs * (8 / 2^24)
            a1 = smp.tile([128, 1], F32, tag="a1")
            nc.vector.tensor_scalar(out=a1, in0=dnegs, scalar1=8.0 / 16777216.0,
                                    scalar2=8.0 * SHIFT_C,
                                    op0=OP.mult, op1=OP.add)
            nc.vector.tensor_tensor(out=a1, in0=a1, in1=wl, op=OP.add)
            nc.vector.tensor_tensor(out=a1, in0=a1, in1=tsh, op=OP.subtract)

            pss = psb.tile([1, 1], F32, tag="psum_out")
            nc.tensor.matmul(pss, lhsT=a1, rhs=ones, start=True, stop=True)
            outt = smp.tile([1, 1], F32, tag="outt")
            nc.vector.tensor_copy(out=outt, in_=pss)
            nc.sync.dma_start(out=out_d.ap(), in_=outt)

    nc.compile()
    return nc


_NC_CACHE = {}


def get_nc():
    if "nc" not in _NC_CACHE:
        _NC_CACHE["nc"] = build_nc()
    return _NC_CACHE["nc"]


def make_in_maps(x, target):
    x = np.ascontiguousarray(np.asarray(x, dtype=np.float32))
    tgt = np.asarray(target).astype(np.int64)
    assert x.shape == (B, C) and tgt.shape == (B,)

    xbf = np.ascontiguousarray(x.astype(ml_dtypes.bfloat16))

    # members[c] = sorted rows of class c (exactly M each)
    order = np.argsort(tgt, kind="stable")
    members = order.reshape(C, M).astype(np.int64)

    w8row = np.array([8.0, 2.0] + [1.0] * (M - 2), dtype=np.float32)
    w7row = np.array([2.0] + [1.0] * (M - 2), dtype=np.float32)
    w8_full = np.ascontiguousarray(np.broadcast_to(w8row, (128, M)))
    w7_full = np.ascontiguousarray(np.broadcast_to(w7row, (128, M - 1)))

    in_maps = []
    for k in range(NCORES):
        ck = np.arange(k * CCL, (k + 1) * CCL)
        mem_k = members[ck]                      # [128, M]
        xmem = np.ascontiguousarray(
            xbf[mem_k.T.reshape(-1)])            # [M*128, C] bf16, m-major
        dxm = np.ascontiguousarray(
            x[mem_k, ck[:, None]].astype(np.float32))   # [128, M]
        encm = np.ascontiguousarray(
            (float(B) - 1.0 - mem_k.T).T.astype(np.float32))  # [128, M]
        eqmcs = np.zeros((128, C), dtype=np.float32)
        eqmcs[np.arange(CCL), ck] = MASKC
        in_maps.append({
            "xbf": xbf,
            "xmem": xmem,
            "eqmcs": eqmcs,
            "encm": encm,
            "dxm": dxm,
            "w8": w8_full,
            "w7": w7_full,
        })
    return in_maps


def kernel(x, target):
    nc = get_nc()
    in_maps = make_in_maps(x, target)
    res = run_bass_kernel_spmd(nc, in_maps, core_ids=list(range(NCORES)))
    total = sum(float(res.results[k]["partial"][0, 0]) for k in range(NCORES))
    return np.float32(total / B)


# revision 12
# speedup vs baseline: 1.1413x; 1.1413x over previous
"""Trainium2 Bass kernel for the hard-negative-mining set loss (v7).

Key structural changes vs v6:
  * mining rows == member rows: core k mines the M=8 member rows of its
    128 owned classes (any row->core partition works since keys carry
    the true global row id and combine by max). xloc/xmem unify -> one
    2MB bf16 load instead of 12MB f32.
  * no combo tensor: under this sharding the same-class mask column is
    the partition's own class for every tile, so a single eqmcs
    (-2^24 at col g_p) is added ONCE to the maxed key accumulator; the
    enc tiebreak rides the per-partition tensor_scalar scalars.
  * tail: rsum_m = sum exp(mem_m + mem_0 + neg - 14) computed by a DVE
    add + fused ACT exp-accumulate (no Pool multiplies / reduces).
  * negrow gathered from a bf16 copy of x in one indirect DMA.
"""

import ml_dtypes
import numpy as np

import concourse.bass as bass
import concourse.bacc as bacc
import concourse.tile as tile
from concourse import mybir
from concourse import bass_isa
from concourse.bass_utils import run_bass_kernel_spmd
from concourse.tile import add_dep_helper

B, C = 8192, 1024
NCORES = 8
CCL = C // NCORES     # 128 classes owned per core
M = B // C            # 8 members per class

SHIFT_A = 10.0        # mining softmax shift
SHIFT_C = 14.0        # summed-logits softmax shift
QSCALE = 140.0        # log-prob quantization: 1/140 nat resolution
SCALE = QSCALE * 8192.0            # 1146880.0
M2 = 1.5 * (2.0 ** 36)             # magic: ulp(M2) = 8192
M2C = M2 + SCALE * SHIFT_A         # exact multiple of 8192
MASKC = -16777216.0                # -2^24 same-class exclusion
F32 = mybir.dt.float32
BF16 = mybir.dt.bfloat16
I32 = mybir.dt.int32
OP = mybir.AluOpType
AF = mybir.ActivationFunctionType
AX = mybir.AxisListType


def build_nc():
    nc = bacc.Bacc("TRN2", target_bir_lowering=False, debug=False,
                   num_devices=NCORES)

    xbf_d = nc.dram_tensor("xbf", [B, C], BF16, kind="ExternalInput")
    xmem_d = nc.dram_tensor("xmem", [M * 128, C], BF16, kind="ExternalInput")
    eqmcs_d = nc.dram_tensor("eqmcs", [128, C], F32, kind="ExternalInput")
    encm_d = nc.dram_tensor("encm", [128, M], F32, kind="ExternalInput")
    dxm_d = nc.dram_tensor("dxm", [128, M], F32, kind="ExternalInput")
    w8_d = nc.dram_tensor("w8", [128, M], F32, kind="ExternalInput")
    w7_d = nc.dram_tensor("w7", [128, M - 1], F32, kind="ExternalInput")
    out_d = nc.dram_tensor("partial", [1, 1], F32, kind="ExternalOutput")

    cc_in = nc.dram_tensor("cc_in", [1, C], F32)
    cc_out = nc.dram_tensor("cc_out", [1, C], F32)

    with tile.TileContext(nc) as tc:
        with (
            tc.tile_pool(name="persist", bufs=1) as pp,
            tc.tile_pool(name="rscr", bufs=3) as rp,
            tc.tile_pool(name="kscr", bufs=3) as kp,
            tc.tile_pool(name="tscr", bufs=2) as tp,
            tc.tile_pool(name="small", bufs=6) as smp,
            tc.tile_pool(name="psB", bufs=1, space="PSUM") as psb,
        ):
            # ---------- input DMAs ----------
            xm = []
            for m in range(M):
                xt = pp.tile([128, C], BF16, tag=f"xm{m}")
                nc.gpsimd.dma_start(out=xt, in_=xmem_d.ap()[m * 128:(m + 1) * 128, :])
                xm.append(xt)
            # aux DMAs on the scalar HWDGE queue: keeps the sync queue free
            # for the runtime's ACT-table staging (gates the first exp)
            eqmcs = pp.tile([128, C], F32, tag="eqmcs")
            nc.scalar.dma_start(out=eqmcs, in_=eqmcs_d.ap())
            encm = pp.tile([128, M], F32, tag="encm")
            nc.scalar.dma_start(out=encm, in_=encm_d.ap())
            dxm = pp.tile([128, M], F32, tag="dxm")
            nc.scalar.dma_start(out=dxm, in_=dxm_d.ap())
            w8 = pp.tile([128, M], F32, tag="w8")
            nc.scalar.dma_start(out=w8, in_=w8_d.ap())
            w7 = pp.tile([128, M - 1], F32, tag="w7")
            nc.scalar.dma_start(out=w7, in_=w7_d.ap())

            ones = pp.tile([128, 1], F32, tag="ones")
            nc.vector.memset(ones, 1.0)
            shA = pp.tile([128, 1], F32, tag="shA")
            nc.vector.memset(shA, -SHIFT_A)
            shC = pp.tile([128, 1], F32, tag="shC")
            nc.vector.memset(shC, -SHIFT_C)

            # ---------- mining: packed-key build ----------
            # rs[:, m] = sum_c exp(xm_m - SHIFT_A)
            dumpA = pp.tile([128, C], BF16, tag="dumpA")
            rscat = smp.tile([128, M], F32, tag="rscat")
            for m in range(M):
                nc.scalar.activation(out=dumpA, in_=xm[m], func=AF.Exp,
                                     bias=shA, scale=1.0,
                                     accum_out=rscat[:, m:m + 1])
            lrcat = smp.tile([128, M], F32, tag="lrcat")
            ln_ins = nc.scalar.activation(out=lrcat, in_=rscat, func=AF.Ln)
            # b_m = f32(SCALE*lr + M2C): multiple of 8192 (carries lnrsum)
            btcat = smp.tile([128, M], F32, tag="btcat")
            nc.vector.tensor_scalar(out=btcat, in0=lrcat, scalar1=SCALE,
                                    scalar2=M2C, op0=OP.mult, op1=OP.add)
            bt = [btcat[:, m:m + 1] for m in range(M)]
            en = [encm[:, m:m + 1] for m in range(M)]
            # r_m = Relu(-SCALE*x + b_m) = b_m + 8192*q (fp32 rounds @8192)
            # K_m = (r_m - b_m) + enc_m ; Kacc = max_m K_m ; then += eqmcs
            # two max-accumulator chains (m 0-3 / 4-7) to shorten the
            # serial dependency behind the relu stream
            KaccA = pp.tile([128, C], F32, tag="KaccA")
            KaccB = pp.tile([128, C], F32, tag="KaccB")
            acc = {0: KaccA, 4: KaccB}
            relu_ins = None
            HC = C // 2
            for m in range(M):
                rt = rp.tile([128, C], F32, tag="relu")
                relu_ins = nc.scalar.activation(out=rt, in_=xm[m],
                                                func=AF.Relu,
                                                bias=bt[m], scale=-SCALE)
                if m == 0:
                    # pin ACT queue order: the Ln must precede the relus
                    add_dep_helper(relu_ins.ins, ln_ins.ins, sync=False)
                dst = KaccA if m < 4 else KaccB
                if m in acc:
                    nc.vector.tensor_scalar(
                        out=dst, in0=rt, scalar1=bt[m], op0=OP.subtract,
                        scalar2=en[m], op1=OP.add)
                else:
                    kt = kp.tile([128, C], F32, tag="kt")
                    nc.vector.tensor_scalar(
                        out=kt, in0=rt, scalar1=bt[m], op0=OP.subtract,
                        scalar2=en[m], op1=OP.add)
                    nc.vector.scalar_tensor_tensor(
                        out=dst, in0=dst, scalar=0.0, op0=OP.add,
                        in1=kt, op1=OP.max)
            Kacc = pp.tile([128, C], F32, tag="Kacc")
            nc.vector.scalar_tensor_tensor(
                out=Kacc, in0=KaccA, scalar=0.0, op0=OP.add,
                in1=KaccB, op1=OP.max)
            # same-class exclusion, applied once post-max
            last_k = nc.vector.tensor_tensor(out=Kacc, in0=Kacc, in1=eqmcs,
                                             op=OP.add)

            # local per-class max over partitions (split halves so the
            # cc_in store of half 0 overlaps the reduce of half 1), then
            # 4KB AllToAll: my out chunk n = core n's local max for MY
            # 128 classes
            Kpar = pp.tile([128, C], F32, tag="Kpar")
            for h in (0, 1):
                lo, hi = h * HC, (h + 1) * HC
                nc.gpsimd.partition_all_reduce(
                    out_ap=Kpar[:, lo:hi], in_ap=Kacc[:, lo:hi], channels=128,
                    reduce_op=bass_isa.ReduceOp.max)
                nc.scalar.dma_start(out=cc_in.ap()[0:1, lo:hi],
                                    in_=Kpar[0:1, lo:hi])
            nc.gpsimd.collective_compute(
                "AllToAll", OP.bypass,
                replica_groups=[list(range(NCORES))],
                ins=[cc_in.ap().opt()], outs=[cc_out.ap().opt()])

            # ---------- overlapped with barrier/exchange ----------
            # E_m = exp(mem_m + mem_0 - SHIFT_C), bf16, for the tail dots
            Em = []
            for m in range(1, M):
                st = rp.tile([128, C], F32, tag="Sm")
                s_ins = nc.vector.scalar_tensor_tensor(
                    out=st, in0=xm[m], scalar=0.0, op0=OP.add,
                    in1=xm[0], op1=OP.add)
                et = pp.tile([128, C], BF16, tag=f"Em{m}")
                e_ins = nc.scalar.activation(out=et, in_=st, func=AF.Exp,
                                             bias=shC, scale=1.0)
                Em.append(et)
                if m == 1:
                    add_dep_helper(s_ins.ins, last_k.ins, sync=False)
                    add_dep_helper(e_ins.ins, relu_ins.ins, sync=False)
            # tsh = sum_m w8_m * dxm_m  (host-gathered target logits)
            scr8 = smp.tile([128, M], F32, tag="scr8")
            tsh = smp.tile([128, 1], F32, tag="tsh")
            nc.vector.scalar_tensor_tensor(
                out=scr8, in0=dxm, scalar=1.0, in1=w8,
                op0=OP.mult, op1=OP.mult, accum_out=tsh)

            # ---------- combine + decode + neg gather ----------
            # cc_out chunk s = core s's local max for MY classes; load it
            # as [128, 8] (partition p = my class p, col s = sender) and
            # max-reduce over the free dim -> global key per owned class
            k8t = smp.tile([CCL, NCORES], F32, tag="k8t")
            k8src = bass.AP(tensor=cc_out.ap().tensor, offset=0,
                            ap=[[1, CCL], [CCL, NCORES]])
            nc.scalar.dma_start(out=k8t, in_=k8src)
            gk = smp.tile([128, 1], F32, tag="gk")
            nc.vector.tensor_reduce(out=gk, in_=k8t, axis=AX.X, op=OP.max)
            # decode: key is an exact integer in f32; row = 8191 - (key & 8191)
            gi = smp.tile([128, 1], I32, tag="gi")
            nc.vector.tensor_copy(out=gi, in_=gk)
            enc_i = smp.tile([128, 1], I32, tag="enc_i")
            nc.vector.tensor_scalar(out=enc_i, in0=gi, scalar1=8191,
                                    scalar2=None, op0=OP.bitwise_and)
            rowi = smp.tile([128, 1], I32, tag="rowi")
            nc.vector.tensor_scalar(out=rowi, in0=enc_i, scalar1=-1,
                                    scalar2=8191, op0=OP.mult, op1=OP.add)
            # ---------- factored CE tail ----------
            # N = exp(neg); rsum_m = sum_c E_m * N   (bf16 dots, f32 accum)
            # gather + N-exp in column halves: the exp of half 0 overlaps
            # the transfer of half 1
            negrow = pp.tile([128, C], BF16, tag="negrow")
            Nt = pp.tile([128, C], BF16, tag="Nt")
            n_ins = None
            for h in (0, 1):
                lo, hi = h * HC, (h + 1) * HC
                nc.gpsimd.indirect_dma_start(
                    out=negrow[:, lo:hi], out_offset=None,
                    in_=xbf_d.ap(), element_offset=lo,
                    in_offset=bass.IndirectOffsetOnAxis(ap=rowi[:, 0:1],
                                                        axis=0))
                n_ins = nc.scalar.activation(out=Nt[:, lo:hi],
                                             in_=negrow[:, lo:hi],
                                             func=AF.Exp, bias=0.0, scale=1.0)
                if h == 0:
                    add_dep_helper(n_ins.ins, e_ins.ins, sync=False)
            rscat3 = smp.tile([128, M - 1], F32, tag="rscat3")
            dumpC = tp.tile([128, C], BF16, tag="dumpC")
            for m in range(1, M):
                nc.vector.scalar_tensor_tensor(
                    out=dumpC, in0=Em[m - 1], scalar=1.0, op0=OP.mult,
                    in1=Nt, op1=OP.mult,
                    accum_out=rscat3[:, m - 1:m])
            # dneg_scaled = -2^24 * neg[p, class(p)] via eqmcs diag dot
            scrC = tp.tile([128, C], F32, tag="scrC")
            dnegs = smp.tile([128, 1], F32, tag="dnegs")
            nc.vector.scalar_tensor_tensor(out=scrC, in0=negrow, scalar=1.0,
                                           in1=eqmcs, op0=OP.mult, op1=OP.mult,
                                           accum_out=dnegs)
            lcat = smp.tile([128, M - 1], F32, tag="lcat")
            lcat_ins = nc.scalar.activation(out=lcat, in_=rscat3, func=AF.Ln)
            add_dep_helper(lcat_ins.ins, n_ins.ins, sync=False)
            scr7 = smp.tile([128, M - 1], F32, tag="scr7")
            wl = smp.tile([128, 1], F32, tag="wl")
            nc.vector.scalar_tensor_tensor(out=scr7, in0=lcat, scalar=1.0,
                                           in1=w7, op0=OP.mult, op1=OP.mult,
                                           accum_out=wl)
            # loss_p = wl + 8*SHIFT_C - tsh - 8*dneg
            #        = wl + 8*SHIFT_C - tsh + dnegs * (8 / 2^24)
            a1 = smp.tile([128, 1], F32, tag="a1")
            nc.vector.tensor_scalar(out=a1, in0=dnegs, scalar1=8.0 / 16777216.0,
                                    scalar2=8.0 * SHIFT_C,
                                    op0=OP.mult, op1=OP.add)
            nc.vector.tensor_tensor(out=a1, in0=a1, in1=wl, op=OP.add)
            nc.vector.tensor_tensor(out=a1, in0=a1, in1=tsh, op=OP.subtract)

            pss = psb.tile([1, 1], F32, tag="psum_out")
            nc.tensor.matmul(pss, lhsT=a1, rhs=ones, start=True, stop=True)
            outt = smp.tile([1, 1], F32, tag="outt")
            nc.vector.tensor_copy(out=outt, in_=pss)
            nc.sync.dma_start(out=out_d.ap(), in_=outt)

    nc.compile()
    return nc


_NC_CACHE = {}


def get_nc():
    if "nc" not in _NC_CACHE:
        _NC_CACHE["nc"] = build_nc()
    return _NC_CACHE["nc"]


def make_in_maps(x, target):
    x = np.ascontiguousarray(np.asarray(x, dtype=np.float32))
    tgt = np.asarray(target).astype(np.int64)
    assert x.shape == (B, C) and tgt.shape == (B,)

    xbf = np.ascontiguousarray(x.astype(ml_dtypes.bfloat16))

    # members[c] = sorted rows of class c (exactly M each)
    order = np.argsort(tgt, kind="stable")
    members = order.reshape(C, M).astype(np.int64)

    w8row = np.array([8.0, 2.0] + [1.0] * (M - 2), dtype=np.float32)
    w7row = np.array([2.0] + [1.0] * (M - 2), dtype=np.float32)
    w8_full = np.ascontiguousarray(np.broadcast_to(w8row, (128, M)))
    w7_full = np.ascontiguousarray(np.broadcast_to(w7row, (128, M - 1)))

    in_maps = []
    for k in range(NCORES):
        ck = np.arange(k * CCL, (k + 1) * CCL)
        mem_k = members[ck]                      # [128, M]
        xmem = np.ascontiguousarray(
            xbf[mem_k.T.reshape(-1)])            # [M*128, C] bf16, m-major
        dxm = np.ascontiguousarray(
            x[mem_k, ck[:, None]].astype(np.float32))   # [128, M]
        encm = np.ascontiguousarray(
            (float(B) - 1.0 - mem_k.T).T.astype(np.float32))  # [128, M]
        eqmcs = np.zeros((128, C), dtype=np.float32)
        eqmcs[np.arange(CCL), ck] = MASKC
        in_maps.append({
            "xbf": xbf,
            "xmem": xmem,
            "eqmcs": eqmcs,
            "encm": encm,
            "dxm": dxm,
            "w8": w8_full,
            "w7": w7_full,
        })
    return in_maps


def kernel(x, target):
    nc = get_nc()
    in_maps = make_in_maps(x, target)
    res = run_bass_kernel_spmd(nc, in_maps, core_ids=list(range(NCORES)))
    total = sum(float(res.results[k]["partial"][0, 0]) for k in range(NCORES))
    return np.float32(total / B)
